# revision 3
# baseline (speedup 1.0000x reference)
"""BiologicallyInformedAttention TRN2 kernel (8 NeuronCores, axon/PJRT).

Sharding: B*H = 32 (batch, head) pairs over 8 cores -> core c handles batch
c//2, heads (c%2)*4 .. +4. Projection weights are column-sliced per core; x is
transposed host-side so every matmul contracts over the partition dim.

v2: ACT-bound pipeline. Attention runs per-head in 256 fine steps (p, qh, h,
kt); each step = scores mm [128,1024] psum (ring bufs=2 -> 2-step lead), one
exp on ACT (the bottleneck: 256 x ~1038ns), 2 AV accumulate mms. The identity
prior is a bf16 PE accumulate-matmul (pw8i_bf16 stationary x eye_bf16), so no
DVE sits between scores and exp. Projections, softmax-normalize, and the
output projection are sliced into <=1us "extras" interleaved into PE slack so
ACT never starves. PSUM: sc 2x2 + av 2x1 + proj/out 2x1 = 8 banks.
"""
import numpy as np
from collections import defaultdict
from contextlib import ExitStack

import concourse.bacc as bacc
import concourse.tile as tile
from concourse import mybir
from concourse.bass_utils import run_bass_kernel_spmd

B, S, D, H, DH = 4, 2048, 512, 8, 64
HPC = H // 2          # heads per core = 4
W_COLS = HPC * DH     # 256 per-core projection columns
N_CORES = 8

f32 = mybir.dt.float32
f32r = mybir.dt.float32r
bf16 = mybir.dt.bfloat16
AF = mybir.ActivationFunctionType

_BUILT = {}


def _build(repeat=1):
    nc = bacc.Bacc("TRN2", target_bir_lowering=False)

    xT_d = nc.declare_dram_parameter("xT", [D, S], f32r, isOutput=False)
    wq_d = nc.declare_dram_parameter("wq", [D, W_COLS], f32r, isOutput=False)
    wk_d = nc.declare_dram_parameter("wk", [D, W_COLS], f32r, isOutput=False)
    wv_d = nc.declare_dram_parameter("wv", [D, W_COLS], f32r, isOutput=False)
    wo_d = nc.declare_dram_parameter("wo", [W_COLS, DH], f32r, isOutput=False)
    bq_d = nc.declare_dram_parameter("bq", [W_COLS, 1], f32, isOutput=False)
    bk_d = nc.declare_dram_parameter("bk", [W_COLS, 1], f32, isOutput=False)
    bv_d = nc.declare_dram_parameter("bv", [1, W_COLS], f32r, isOutput=False)
    pw8i_d = nc.declare_dram_parameter("pw8i", [128, 128], bf16, isOutput=False)
    eye_d = nc.declare_dram_parameter("eye16", [128, 128], bf16, isOutput=False)
    ones_row_d = nc.declare_dram_parameter("ones_row", [1, 128], f32r, isOutput=False)
    ones_blk_d = nc.declare_dram_parameter("ones_blk", [128, 8], f32r, isOutput=False)
    outT_d = nc.declare_dram_parameter("outT", [DH, S], f32, isOutput=True)

    with tile.TileContext(nc) as tc, ExitStack() as ctx:
        cp = ctx.enter_context(tc.tile_pool(name="cp", bufs=1))

        # ---------- persistent tiles ----------
        xr = [cp.tile([128, S], f32r, tag=f"xr{i}", name=f"xr{i}") for i in range(4)]
        wqr = [cp.tile([128, W_COLS], f32r, tag=f"wqr{i}", name=f"wqr{i}") for i in range(4)]
        wkr = [cp.tile([128, W_COLS], f32r, tag=f"wkr{i}", name=f"wkr{i}") for i in range(4)]
        wvr = [cp.tile([128, W_COLS], f32r, tag=f"wvr{i}", name=f"wvr{i}") for i in range(4)]
        wor = cp.tile([DH, W_COLS], f32r, tag="wor", name="wor")
        bq_t = cp.tile([128, 2], f32, tag="bq", name="bq")
        bk_t = cp.tile([128, 2], f32, tag="bk", name="bk")
        bvr = cp.tile([1, W_COLS], f32r, tag="bvr", name="bvr")
        ones_col = cp.tile([1, 128], f32r, tag="ones_col", name="ones_col")
        ones_blk = cp.tile([128, 8], f32r, tag="ones_blk", name="ones_blk")
        pw8i = cp.tile([128, 128], bf16, tag="pw8i", name="pw8i")
        eye16 = cp.tile([128, 128], bf16, tag="eye16", name="eye16")
        qTr = [cp.tile([128, S], f32r, tag=f"qTr{p}", name=f"qTr{p}") for p in range(2)]
        kTr = [cp.tile([128, S], f32r, tag=f"kTr{p}", name=f"kTr{p}") for p in range(2)]
        v_aug = [cp.tile([128, HPC * 66], f32r, tag=f"va{st}", name=f"va{st}") for st in range(16)]
        attnT = [cp.tile([DH, S], f32r, tag=f"at{h}", name=f"at{h}") for h in range(HPC)]
        outT_s = cp.tile([DH, S], f32, tag="outT", name="outT")

        # ---------- loads (all f32r DMA-direct) ----------
        for di in range(4):
            nc.sync.dma_start(wqr[di][:], wq_d[di * 128:(di + 1) * 128, :])
            nc.sync.dma_start(wkr[di][:], wk_d[di * 128:(di + 1) * 128, :])
        for sc4 in range(4):
            s0 = sc4 * 512
            for di in range(4):
                nc.sync.dma_start(xr[di][:, s0:s0 + 512],
                                  xT_d[di * 128:(di + 1) * 128, s0:s0 + 512])
            if sc4 == 1:
                for di in range(4):
                    nc.sync.dma_start(wvr[di][:], wv_d[di * 128:(di + 1) * 128, :])
        for h in range(HPC):
            nc.sync.dma_start(wor[:, h * DH:(h + 1) * DH],
                              wo_d[h * DH:(h + 1) * DH, :])
        for ht in range(2):
            nc.sync.dma_start(bq_t[:, ht:ht + 1], bq_d[ht * 128:(ht + 1) * 128, :])
            nc.sync.dma_start(bk_t[:, ht:ht + 1], bk_d[ht * 128:(ht + 1) * 128, :])
        nc.sync.dma_start(bvr[:], bv_d[:])
        nc.sync.dma_start(ones_col[:], ones_row_d[:])
        nc.sync.dma_start(ones_blk[:], ones_blk_d[:])
        nc.sync.dma_start(pw8i[:], pw8i_d[:])
        nc.sync.dma_start(eye16[:], eye_d[:])

        with tc.tile_pool(name="scp", bufs=2, space="PSUM") as scp, \
             tc.tile_pool(name="avp", bufs=1, space="PSUM") as avp, \
             tc.tile_pool(name="pjp", bufs=2, space="PSUM") as pjp, \
             tc.tile_pool(name="etp", bufs=4) as etp, \
             tc.tile_pool(name="nrm", bufs=2) as nrm:

            # ---- extras: small closures interleaved into attention steps ----
            def proj_qk_slice(ht, sc4, which):
                """One 512-col q-or-k projection slice: 4 mms + bias evac."""
                s0 = sc4 * 512
                wr, bias_t, dst = ((wqr, bq_t, qTr) if which == 0
                                   else (wkr, bk_t, kTr))
                pt = pjp.tile([128, 512], f32, tag="pj", name="pj")
                for di in range(4):
                    nc.tensor.matmul(
                        pt[:],
                        wr[di][:, ht * 128:(ht + 1) * 128],
                        xr[di][:, s0:s0 + 512],
                        start=(di == 0), stop=(di == 3))
                nc.vector.tensor_scalar_add(
                    dst[ht][:, s0:s0 + 512], pt[:], bias_t[:, ht:ht + 1])

            def proj_v(st):
                pv = pjp.tile([128, W_COLS], f32, tag="pj", name="pv")
                for di in range(4):
                    nc.tensor.matmul(pv[:],
                                     xr[di][:, st * 128:(st + 1) * 128],
                                     wvr[di][:],
                                     start=(di == 0), stop=False)
                nc.tensor.matmul(pv[:], ones_col[:], bvr[:],
                                 start=False, stop=True)
                va = v_aug[st][:].rearrange("p (h c) -> p h c", c=66)
                nc.vector.tensor_copy(
                    va[:, :, 0:DH],
                    pv[:].rearrange("p (h c) -> p h c", c=DH))
                nc.vector.tensor_copy(
                    va[:, :, DH:66],
                    ones_blk[:].rearrange("p (h c) -> p h c", c=2))

            def out_proj_slice(qh, sc4):
                s0 = sc4 * 512
                po = pjp.tile([DH, 512], f32, tag="pj", name="po")
                for h in range(HPC):
                    nc.tensor.matmul(po[:],
                                     wor[:, h * DH:(h + 1) * DH],
                                     attnT[h][:, s0:s0 + 512],
                                     start=(h == 0), stop=(h == HPC - 1))
                nc.vector.tensor_copy(outT_s[:, s0:s0 + 512], po[:])
                nc.sync.dma_start(outT_d[:, s0:s0 + 512], outT_s[:, s0:s0 + 512])

            for _rep in range(repeat):
                # head phase: enough q/k/v for the first head's 16 steps
                for sc4 in (0, 1):
                    proj_qk_slice(0, sc4, 0)
                    proj_qk_slice(0, sc4, 1)
                for st in range(4):
                    proj_v(st)

                # static extras schedule: step -> list of closures, run AFTER
                # that step's S/E/AV are emitted. A producer extra must sit at
                # a step strictly before its first consumer's emission step.
                sched = defaultdict(list)
                sched[0].append(lambda: proj_qk_slice(0, 2, 1))   # kT blk2 (use: kt8)
                sched[6].append(lambda: proj_qk_slice(0, 3, 1))   # kT blk3 (use: kt12)
                vs = {4: 1, 5: 1, 6: 2, 7: 3, 8: 4, 9: 5, 10: 6, 11: 7,
                      12: 8, 13: 9, 14: 10, 15: 12}               # v(st) emit step
                for st, s in vs.items():
                    sched[s].append(lambda st=st: proj_v(st))
                # qT blocks 2,3 (needed at qh=1 emission, step 32)
                sched[16].append(lambda: proj_qk_slice(0, 2, 0))
                sched[18].append(lambda: proj_qk_slice(0, 3, 0))
                # pair-1 projections, needed by emission step 128
                s = 20
                for sc4 in range(4):
                    for which in (1, 0):
                        sched[s].append(
                            lambda sc4=sc4, which=which: proj_qk_slice(1, sc4, which))
                        s += 3

                def norm_head(base_step, p, qh, h, av_q0, av_q1):
                    """Softmax-normalize head (p,qh,h): the two av-psum
                    evacuations are emitted inline (they must precede the next
                    head's first AV write into the same psum slots); the rest
                    is queued into following steps."""
                    hh = 2 * p + h
                    q0 = qh * 1024
                    avs = nrm.tile([66, 1024], f32, tag="avs", name="avs")
                    recip = nrm.tile([1, 1024], f32, tag="recip", name="recip")
                    rB = nrm.tile([DH, 1024], f32, tag="rB", name="rB")
                    nc.vector.tensor_copy(avs[:, 0:512], av_q0[:])
                    nc.vector.tensor_copy(avs[:, 512:1024], av_q1[:])
                    ops = [
                        lambda: nc.vector.reciprocal_approx_fast(
                            recip[:], avs[DH:DH + 1, :]),
                        lambda: nc.gpsimd.partition_broadcast(rB[:], recip[:]),
                        lambda: nc.vector.tensor_mul(
                            attnT[hh][:, q0:q0 + 1024], avs[0:DH, :], rB[:]),
                    ]
                    for i, op in enumerate(ops):
                        sched[base_step + 1 + 2 * i].append(op)
                    # output projection once the last head of a qh (p==1) done
                    if p == 1 and h == 1:
                        for i, sc4 in enumerate((2 * qh, 2 * qh + 1)):
                            sched[base_step + 7 + 2 * i].append(
                                lambda qh=qh, sc4=sc4: out_proj_slice(qh, sc4))

                step = 0
                for p in (0, 1):
                    for qh in (0, 1):
                        q0 = qh * 1024
                        for h in (0, 1):
                            base = 64 * h
                            av_q0 = avp.tile([66, 512], f32, tag="avq0", name="avq0")
                            av_q1 = avp.tile([66, 512], f32, tag="avq1", name="avq1")
                            hh66 = (2 * p + h) * 66
                            for kt in range(16):
                                k0 = kt * 128
                                off = k0 - q0
                                sc_t = scp.tile([128, 1024], f32, tag="sc", name="sc")
                                for qc in range(2):
                                    qq = qc * 512
                                    nc.tensor.matmul(
                                        sc_t[:, qq:qq + 512],
                                        kTr[p][base:base + 64, k0:k0 + 128],
                                        qTr[p][base:base + 64, q0 + qq:q0 + qq + 512],
                                        start=True, stop=True)
                                if 0 <= off < 1024:
                                    nc.tensor.matmul(
                                        sc_t[:, off:off + 128],
                                        pw8i[:], eye16[:],
                                        start=False, stop=True)
                                et = etp.tile([128, 1024], f32r, tag="et", name="et")
                                nc.scalar.activation(et[:], sc_t[:], AF.Exp, scale=0.125)
                                for qc, av in ((0, av_q0), (1, av_q1)):
                                    qq = qc * 512
                                    nc.tensor.matmul(
                                        av[:],
                                        v_aug[kt][:, hh66:hh66 + 66],
                                        et[:, qq:qq + 512],
                                        start=(kt == 0), stop=(kt == 15))
                                for fn in sched.pop(step, []):
                                    fn()
                                step += 1
                            norm_head(step, p, qh, h, av_q0, av_q1)
                # trailing extras (last head's norm + out_proj)
                for st2 in sorted(sched):
                    for fn in sched.pop(st2, []):
                        fn()

    nc.finalize()
    return nc


def _get_nc(repeat=1):
    if repeat not in _BUILT:
        _BUILT[repeat] = _build(repeat)
    return _BUILT[repeat]


def _make_in_maps(x, Wq, bq, Wk, bk, Wv, bv, Wo, bo, prior_weight):
    import ml_dtypes
    pw8i = ((8.0 * float(prior_weight[0])) * np.eye(128, dtype=np.float32)
            ).astype(ml_dtypes.bfloat16)
    eye16 = np.eye(128, dtype=np.float32).astype(ml_dtypes.bfloat16)
    ones_row = np.ones((1, 128), np.float32)
    ones_blk = np.ones((128, 8), np.float32)
    xT = [np.ascontiguousarray(x[b].T) for b in range(B)]
    in_maps = []
    for c in range(N_CORES):
        b, half = c // 2, c % 2
        cs = slice(half * W_COLS, (half + 1) * W_COLS)
        in_maps.append({
            "xT": xT[b],
            "wq": np.ascontiguousarray(Wq[:, cs]),
            "wk": np.ascontiguousarray(Wk[:, cs]),
            "wv": np.ascontiguousarray(Wv[:, cs]),
            "wo": np.ascontiguousarray(Wo[cs, :]),
            "bq": np.ascontiguousarray(bq[cs].reshape(W_COLS, 1)),
            "bk": np.ascontiguousarray(bk[cs].reshape(W_COLS, 1)),
            "bv": np.ascontiguousarray(bv[cs].reshape(1, W_COLS)),
            "pw8i": pw8i,
            "eye16": eye16,
            "ones_row": ones_row,
            "ones_blk": ones_blk,
        })
    return in_maps


def run(inputs, trace=False, trace_cores=None):
    """Execute on 8 cores; returns (output [B,S,DH] f32, BassKernelResults)."""
    args = {k: np.asarray(v) for k, v in inputs.items()}
    nc = _get_nc()
    in_maps = _make_in_maps(
        args["x"], args["Wq"], args["bq"], args["Wk"], args["bk"],
        args["Wv"], args["bv"], args["Wo"], args["bo"], args["prior_weight"])
    res = run_bass_kernel_spmd(
        nc, in_maps, list(range(N_CORES)), trace=trace,
        **({"trace_cores": trace_cores} if trace_cores else {}))
    bo = args["bo"].astype(np.float32)
    out = np.empty((B, S, DH), np.float32)
    for b in range(B):
        acc = res.results[2 * b]["outT"] + res.results[2 * b + 1]["outT"]
        out[b] = acc.T + bo
    return out, res


def kernel(**inputs) -> np.ndarray:
    out, _ = run(inputs, trace=False)
    return out
